# revision 2
# baseline (speedup 1.0000x reference)
"""Multi-head self-attention (B=4, S=2048, D=1024, H=16, Hd=64) on 8 TRN2 cores.

Sharding: batch x head-half. Core c = (batch b=c//2, head-half hh=c%2) owns
heads hh*8..hh*8+7 of batch b (4 head-pair blocks), all 2048 tokens.

Per core (all matmuls bf16, fp32 psum):
  - Q^T/K^T [512, 2048] projections per head-pair block [128, 2048]
  - V in [token, hd] layout [128-token chunks, 512] (no transposes needed)
  - attention per head-pair: scores S^T [kpos,q] via two K=64 matmuls
    row-packed in PE row-groups (concurrent); exp on ACT (scale 1/8);
    AV + softmax denominator via col-tiled matmuls (V + ones col-groups);
    reciprocal + gpsimd partition-broadcast; normalize -> A^T bf16
  - out-proj partial for ALL 2048 tokens over its 512 A-dims (natural Wo
    rows), fp32 partials -> pairwise ReduceScatter(add) over (2b, 2b+1)
    gives each core the exact fp32 out rows for its 1024 tokens; +bo; DMA.

No AllGather of activations; x read once per batch pair; everything bf16
on the wire (host pre-converts weights/x to bfloat16).
"""
import numpy as np
import ml_dtypes

B, S, D, H, HD = 4, 2048, 1024, 16, 64
N_CORES = 8
T = S                  # tokens per core's batch
NP = 4                 # head-pair blocks per core
NKT = S // 128         # 16 kpos chunks
NQ = 4                 # query quarters of 512
NTC = T // 128         # 16 token chunks

_CACHE = {}


def _build():
    import concourse.bacc as bacc
    import concourse.mybir as mybir
    import concourse.tile as tile
    from collections import deque

    F32 = mybir.dt.float32
    BF16 = mybir.dt.bfloat16
    AF = mybir.ActivationFunctionType

    nc = bacc.Bacc(trn_type="TRN2", target_bir_lowering=False, debug=False,
                   num_devices=N_CORES)

    xb = nc.dram_tensor("xb", [D, T], BF16, kind="ExternalInput")
    wq = nc.dram_tensor("wq", [D, 512], BF16, kind="ExternalInput")
    wk = nc.dram_tensor("wk", [D, 512], BF16, kind="ExternalInput")
    wv = nc.dram_tensor("wv", [D, 512], BF16, kind="ExternalInput")
    wo = nc.dram_tensor("wo", [512, D], BF16, kind="ExternalInput")
    bq = nc.dram_tensor("bq", [512, 1], F32, kind="ExternalInput")
    bk = nc.dram_tensor("bk", [512, 1], F32, kind="ExternalInput")
    bv = nc.dram_tensor("bv", [1, 512], F32, kind="ExternalInput")
    bo = nc.dram_tensor("bo", [1, D], F32, kind="ExternalInput")
    out = nc.dram_tensor("out", [1024, D], F32, kind="ExternalOutput")

    with tile.TileContext(nc) as tc:
        with tc.tile_pool(name="sb", bufs=1) as sb, \
             tc.tile_pool(name="dram", bufs=1, space="DRAM") as dram, \
             tc.tile_pool(name="ps", bufs=1, space="PSUM") as ps:

            # ---------------- prologue: DMAs ------------------------------
            # order: wv first (V-proj is the first consumer), then x chunks,
            # then wq/wk, wo last
            # spread DMA issue across engine queues (~1us issue cost each)
            w_sb = {}
            for wname, wdram, eng in (("wv", wv, nc.scalar),
                                      ("wq", wq, nc.gpsimd),
                                      ("wk", wk, nc.scalar)):
                wt = sb.tile([128, 8 * 512], BF16, tag=wname, name=wname + "_sb")
                wsrc = wdram.ap().rearrange("(k p) m -> p k m", p=128)
                for k in range(0, 8, 2):
                    eng.dma_start(
                        wt[:, k * 512:(k + 2) * 512].rearrange(
                            "p (k m) -> p k m", k=2),
                        wsrc[:, k:k + 2, :])
                w_sb[wname] = wt

            x_sb = sb.tile([128, 8 * T], BF16, tag="xsb", name="x_sb")
            xsrc = xb.ap().rearrange("(k p) m -> p k m", p=128)
            for k in range(8):
                nc.sync.dma_start(x_sb[:, k * T:(k + 1) * T], xsrc[:, k, :])

            wo_sb = sb.tile([128, 4 * D], BF16, tag="wo", name="wo_sb")
            nc.gpsimd.dma_start(
                wo_sb[:].rearrange("p (k m) -> p k m", k=4),
                wo.ap().rearrange("(k p) m -> p k m", p=128))

            bq_sb = sb.tile([128, 4], F32, tag="bq", name="bq_sb")
            nc.sync.dma_start(bq_sb[:].rearrange("p (k m) -> p k m", k=4),
                              bq.ap().rearrange("(k p) m -> p k m", p=128))
            bk_sb = sb.tile([128, 4], F32, tag="bk", name="bk_sb")
            nc.sync.dma_start(bk_sb[:].rearrange("p (k m) -> p k m", k=4),
                              bk.ap().rearrange("(k p) m -> p k m", p=128))

            bvs = sb.tile([1, 512], F32, tag="bvs", name="bvs")
            nc.sync.dma_start(bvs[:], bv[:])
            bvb = sb.tile([128, 512], F32, tag="bvb", name="bvb")
            nc.gpsimd.partition_broadcast(bvb[:], bvs[0:1, :])
            bos = sb.tile([1, D], F32, tag="bos", name="bos")
            nc.sync.dma_start(bos[:], bo[:])
            bob = sb.tile([128, D], F32, tag="bob", name="bob")
            nc.gpsimd.partition_broadcast(bob[:], bos[0:1, :])
            # half-bias: each pair member adds bo/2 to its partial so the
            # ReduceScatter's add completes the bias exactly once
            bobh = sb.tile([128, D], F32, tag="bobh", name="bobh")
            nc.vector.tensor_scalar_mul(bobh[:], bob[:], 0.5)

            ones_av = sb.tile([128, 64], BF16, tag="ones", name="ones_av")
            nc.vector.memset(ones_av[:], 1.0)

            # two merged reduce-scatters (CC cost is latency-dominated):
            # rs X covers token blocks X*4..X*4+3 of each member half
            rsin = [dram.tile([1024, D], BF16, tag=f"rsi{j}", name=f"rsi{j}")
                    for j in range(2)]
            rsout = [dram.tile([512, D], BF16, tag=f"rso{j}", name=f"rso{j}")
                     for j in range(2)]

            v_t = [sb.tile([128, 512], BF16, tag=f"vsb{t}", name=f"vsb{t}")
                   for t in range(NTC)]
            qt = [sb.tile([128, T], BF16, tag=f"qt{p}", name=f"qt{p}")
                  for p in range(NP)]
            kt = [sb.tile([128, T], BF16, tag=f"kt{p}", name=f"kt{p}")
                  for p in range(NP)]
            at = [sb.tile([128, T], BF16, tag=f"at{p}", name=f"at{p}")
                  for p in range(NP)]

            # ---------------- emit helpers --------------------------------
            def v_group(t):
                """V[t*128:(t+1)*128 tokens, all 512 hd] = x_chunk.T @ Wv."""
                holder = {}

                def mk(kc):
                    def go():
                        if "vp" not in holder:
                            holder["vp"] = ps.tile([128, 512], F32,
                                                   tag="aux", bufs=2,
                                                   name=f"vp{t}")
                        nc.tensor.matmul(
                            holder["vp"][:],
                            x_sb[:, kc * T + t * 128:
                                 kc * T + (t + 1) * 128],
                            w_sb["wv"][:, kc * 512:(kc + 1) * 512],
                            start=(kc == 0), stop=(kc == 7))
                    return go

                def fin():
                    nc.vector.tensor_add(v_t[t][:], holder["vp"][:], bvb[:])
                return [mk(kc) for kc in range(8)] + [fin]

            def qk_group(p, wname, st):
                """One 512-token stripe of Q^T/K^T pair-block p."""
                dst = qt[p] if wname == "wq" else kt[p]
                bias = bq_sb if wname == "wq" else bk_sb
                holder = {}

                def mk(kc):
                    def go():
                        if "pp" not in holder:
                            holder["pp"] = ps.tile(
                                [128, 512], F32, tag="aux", bufs=2,
                                name=f"pp{p}{wname}{st}")
                        nc.tensor.matmul(
                            holder["pp"][:],
                            w_sb[wname][:, kc * 512 + p * 128:
                                        kc * 512 + (p + 1) * 128],
                            x_sb[:, kc * T + st * 512:
                                 kc * T + (st + 1) * 512],
                            start=(kc == 0), stop=(kc == 7))
                    return go

                def fin():
                    nc.vector.tensor_scalar_add(
                        dst[:, st * 512:(st + 1) * 512], holder["pp"][:],
                        bias[:, p:p + 1])
                return [mk(kc) for kc in range(8)] + [fin]

            def op_group(t, dh):
                """Out-proj partial: token chunk t, D-half dh."""
                holder = {}

                def mk(ac):
                    def go():
                        if "po" not in holder:
                            holder["po"] = ps.tile(
                                [128, 512], F32, tag="aux", bufs=2,
                                name=f"po{t}_{dh}")
                        nc.tensor.matmul(
                            holder["po"][:],
                            at[ac][:, t * 128:(t + 1) * 128],
                            wo_sb[:, ac * D + dh * 512:
                                  ac * D + dh * 512 + 512],
                            start=(ac == 0), stop=(ac == 3))
                    return go

                def fin():
                    posb = sb.tile([128, 512], BF16, tag="posb", bufs=4,
                                   name=f"posb{t}_{dh}")
                    nc.vector.tensor_add(posb[:], holder["po"][:],
                                         bobh[:, dh * 512:dh * 512 + 512])
                    X, row0 = (t % 8) // 4, (t // 8) * 512 + (t % 4) * 128
                    nc.sync.dma_start(
                        rsin[X][row0:row0 + 128,
                                dh * 512:dh * 512 + 512], posb[:])
                return [mk(ac) for ac in range(4)] + [fin]

            def emit_rs_tail(X):
                nc.gpsimd.collective_compute(
                    "ReduceScatter", mybir.AluOpType.add,
                    replica_groups=[[0, 1], [2, 3], [4, 5], [6, 7]],
                    ins=[rsin[X][:]], outs=[rsout[X][:]])
                for c in range(4):
                    otb = sb.tile([128, D], BF16, tag="otb", bufs=2,
                                  name=f"otb{X}_{c}")
                    nc.sync.dma_start(otb[:], rsout[X][c * 128:(c + 1) * 128, :])
                    otf = sb.tile([128, D], F32, tag="otf", bufs=2,
                                  name=f"otf{X}_{c}")
                    nc.vector.tensor_copy(otf[:], otb[:])
                    nc.sync.dma_start(
                        out[X * 512 + c * 128:X * 512 + (c + 1) * 128, :],
                        otf[:])

            # ---------------- prologue compute ----------------------------
            for t in range(4):
                for cl in v_group(t):
                    cl()
            for st in range(4):
                for cl in qk_group(0, "wq", st) + qk_group(0, "wk", st):
                    cl()

            # tagged job queue: emission of a consumer instruction must come
            # AFTER its producer's emission (the tile framework cannot order
            # a read before a not-yet-emitted write), so attention gates on
            # the producer tags below.
            jobq = deque()          # (tag, closure)
            tagcnt = {}

            def enq(tag, closures):
                for cl in closures:
                    jobq.append((tag, cl))
                tagcnt[tag] = tagcnt.get(tag, 0) + len(closures)

            def pop_jobs(n):
                for _ in range(n):
                    if not jobq:
                        return
                    tag, cl = jobq.popleft()
                    tagcnt[tag] -= 1
                    cl()

            def gate(tag):
                while tagcnt.get(tag, 0) > 0:
                    pop_jobs(1)

            for t in range(4, NTC):
                enq(f"v{t}", v_group(t))
            for p in range(1, NP):
                for wname in ("wq", "wk"):
                    for st in range(4):
                        enq(f"qk{p}", qk_group(p, wname, st))

            # ---------------- attention main loop -------------------------
            pend = {"av": None, "norm": None}
            for quarter in range(NQ):
                q0 = quarter * 512
                for p in range(NP):
                    gate(f"qk{p}")
                    pav = ps.tile([128, 1024], F32, tag="pav", bufs=1,
                                  name=f"pav{quarter}_{p}")

                    def make_av(pav, ptt, ktc, p):
                        def go():
                            st0, sp1 = (ktc == 0), (ktc == NKT - 1)
                            # h0: A rows 0:64 (V), denom rows 64:128 (ones)
                            nc.tensor.matmul(
                                pav[0:64, 0:512],
                                v_t[ktc][:, p * 128:p * 128 + 64],
                                ptt[:, 0:512], start=st0, stop=sp1,
                                tile_position=(0, 0))
                            nc.tensor.matmul(
                                pav[64:128, 0:512], ones_av[:],
                                ptt[:, 0:512], start=st0, stop=sp1,
                                tile_position=(0, 64))
                            # h1: denom rows 0:64 (ones), A rows 64:128 (V)
                            nc.tensor.matmul(
                                pav[0:64, 512:1024], ones_av[:],
                                ptt[:, 512:1024], start=st0, stop=sp1,
                                tile_position=(0, 0))
                            nc.tensor.matmul(
                                pav[64:128, 512:1024],
                                v_t[ktc][:, p * 128 + 64:p * 128 + 128],
                                ptt[:, 512:1024], start=st0, stop=sp1,
                                tile_position=(0, 64))
                        return go

                    def make_norm(pav, p, q0, quarter):
                        def go():
                            # normalize -> A^T. DVE-copy the denominator
                            # rows to partition 0 ([1,x] shifted copy is
                            # HW-verified), evacuate raw A to free pav,
                            # broadcast + reciprocal + aligned muls.
                            dsb = sb.tile([1, 1024], F32, tag="dsb", bufs=2,
                                          name=f"ds{quarter}_{p}")
                            nc.vector.tensor_copy(dsb[0:1, 0:512],
                                                  pav[64:65, 0:512])
                            nc.vector.tensor_copy(dsb[0:1, 512:1024],
                                                  pav[0:1, 512:1024])
                            araw = sb.tile([128, 1024], F32, tag="araw",
                                           bufs=2, name=f"ar{quarter}_{p}")
                            nc.vector.tensor_copy(araw[:], pav[:])
                            bcst = sb.tile([128, 1024], F32, tag="bcs",
                                           bufs=2, name=f"bc{quarter}_{p}")
                            nc.gpsimd.partition_broadcast(bcst[:],
                                                          dsb[0:1, :])
                            rcbt = sb.tile([128, 1024], F32, tag="rcb",
                                           bufs=2, name=f"rc{quarter}_{p}")
                            nc.vector.reciprocal_approx_fast(rcbt[:],
                                                             bcst[:])
                            nc.vector.tensor_mul(at[p][0:64, q0:q0 + 512],
                                                 araw[0:64, 0:512],
                                                 rcbt[0:64, 0:512])
                            nc.vector.tensor_mul(at[p][64:128, q0:q0 + 512],
                                                 araw[64:128, 512:1024],
                                                 rcbt[64:128, 512:1024])
                        return go

                    for ktc in range(NKT):
                        gate(f"v{ktc}")
                        spt = ps.tile([128, 1024], F32, tag="sp", bufs=2,
                                      name=f"sp{quarter}{p}{ktc}")
                        nc.tensor.matmul(
                            spt[:, 0:512],
                            kt[p][0:64, ktc * 128:(ktc + 1) * 128],
                            qt[p][0:64, q0:q0 + 512],
                            start=True, stop=True, tile_position=(0, 0))
                        nc.tensor.matmul(
                            spt[:, 512:1024],
                            kt[p][64:128, ktc * 128:(ktc + 1) * 128],
                            qt[p][64:128, q0:q0 + 512],
                            start=True, stop=True, tile_position=(64, 0))
                        ptt = sb.tile([128, 1024], BF16, tag="pt", bufs=3,
                                      name=f"pt{quarter}{p}{ktc}")
                        nc.scalar.activation(ptt[:], spt[:], AF.Exp,
                                             scale=0.125)
                        # software pipeline: AV of the PREVIOUS iteration
                        # goes behind this iteration's score MMs so the PE
                        # never blocks the next exp's inputs. Jobs pop only
                        # after the pended norm: op-groups read at[] tiles
                        # whose mul is emitted by that norm.
                        if pend["av"] is not None:
                            pend["av"]()
                        if pend["norm"] is not None:
                            pend["norm"]()
                            pend["norm"] = None
                        pop_jobs(2)
                        pend["av"] = make_av(pav, ptt, ktc, p)
                    pend["norm"] = make_norm(pav, p, q0, quarter)
                # end of quarter: enqueue out-proj for its token chunks;
                # rs_j needs chunks t=j (quarters 0-1) and t=8+j (2-3);
                # quarter 3's chunks are handled in the tail
                if quarter < 3:
                    for t in range(quarter * 4, quarter * 4 + 4):
                        for dh in range(2):
                            enq(f"op{t}", op_group(t, dh))
                if quarter == 2:
                    enq("rsA", [lambda: emit_rs_tail(0)])

            # flush the software pipeline
            pend["av"]()
            pend["av"] = None
            pend["norm"]()
            pend["norm"] = None

            # ---------------- drain + tail --------------------------------
            while jobq:
                pop_jobs(1)
            for t in range(12, 16):
                for dh in range(2):
                    for cl in op_group(t, dh):
                        cl()
            emit_rs_tail(1)

    nc.compile()
    return nc


def _get_nc():
    if "nc" not in _CACHE:
        _CACHE["nc"] = _build()
    return _CACHE["nc"]


def _make_in_maps(x, Wq, bq, Wk, bk, Wv, bv, Wo, bo):
    x = np.asarray(x, dtype=np.float32)
    Wq, Wk, Wv, Wo = (np.asarray(w, dtype=np.float32)
                      for w in (Wq, Wk, Wv, Wo))
    bq, bk, bv, bo = (np.asarray(v, dtype=np.float32)
                      for v in (bq, bk, bv, bo))
    bf = ml_dtypes.bfloat16

    xTb = [np.ascontiguousarray(x[b].T).astype(bf) for b in range(B)]
    in_maps = []
    for c in range(N_CORES):
        b, hh = c // 2, c % 2
        cs = slice(hh * 512, (hh + 1) * 512)
        in_maps.append({
            "xb": xTb[b],
            "wq": np.ascontiguousarray(Wq[:, cs]).astype(bf),
            "wk": np.ascontiguousarray(Wk[:, cs]).astype(bf),
            "wv": np.ascontiguousarray(Wv[:, cs]).astype(bf),
            "wo": np.ascontiguousarray(Wo[cs, :]).astype(bf),
            "bq": np.ascontiguousarray(bq[cs].reshape(512, 1)),
            "bk": np.ascontiguousarray(bk[cs].reshape(512, 1)),
            "bv": np.ascontiguousarray(bv[cs].reshape(1, 512)),
            "bo": np.ascontiguousarray(bo.reshape(1, D)),
        })
    return in_maps


def kernel(x, Wq, bq, Wk, bk, Wv, bv, Wo, bo):
    from concourse import bass_utils

    in_maps = _make_in_maps(x, Wq, bq, Wk, bk, Wv, bv, Wo, bo)
    nc = _get_nc()
    res = bass_utils.run_bass_kernel_spmd(nc, in_maps,
                                          core_ids=list(range(N_CORES)))
    _CACHE["last_results"] = res

    outf = np.empty((B, S, D), dtype=np.float32)
    for c in range(N_CORES):
        b, hh = c // 2, c % 2
        outf[b, hh * 1024:(hh + 1) * 1024, :] = res.results[c]["out"]
    return outf
